# revision 15
# baseline (speedup 1.0000x reference)
"""AttentionRetrieval kNN kernel for 8 TRN2 NeuronCores (Bass, raw Block style).

Reference math:
    qp  = query @ Wq.T + bq           (4096, 4096)   [flattened over (D=32, H=128)]
    kp  = support @ Wk.T + bk         (16384, 4096)
    sim = -(|qp|^2 + |kp|^2 - 2 qp@kp.T) / sqrt(128)
    idx, w = top16(sim), softmax(top16 values)

Fused formulation (per-row constants drop out of topk and softmax):
    score[i,j] = sum_d (q_d M) s_d^T [i,j] + g[j]
      M  = (2/sqrt(H)) Wq^T Wk                  (queries projected once, host)
      g  = -|s Wk^T + (bk - bq)|^2 / sqrt(H)    (completed square folds the
                                                 bq-cross-term; global consts drop)
so launch 2 streams the RAW transposed support once — no kpT materialization.

Launch 1 (support sharded 8 x 2048): fp32 (exact) projection with bias
(bk - bq), square + column-sum -> g shard (1 x 2048, 8 KB out per core).

Launch 2 (queries sharded 8 x 512): single-pass float32r matmul
(qm_d stationary, raw supT moving; f32r = RNE-11-bit input rounding at
1 cycle/row — 3x fewer PE rows than an exact hi/lo scheme), + g add, and
per-512-chunk top-8 (DVE max8 + max_index) -> 256 candidates/row.

Host: merge 256 candidates -> top-24, flag rows whose top-17 adjacent gaps
are below the f32r noise bound, exactly rescore flagged rows in f64
(24 dot products each), then top-16 + softmax. Flip rate vs the fp32
reference matches an exact device kernel (~2 rows from fp32 tie noise).
"""
import sys
sys.path.insert(0, "/opt/trn_rl_repo")
import numpy as np
import concourse.bass as bass
from concourse import mybir
from concourse.bass_utils import run_bass_kernel_spmd

f32 = mybir.dt.float32
f32r = mybir.dt.float32r
u16 = mybir.dt.uint16

N_CORES = 8
NQ, NS, D, H = 4096, 16384, 32, 128
DH = D * H
NQ_SH = NQ // N_CORES           # 512
NS_SH = NS // N_CORES           # 2048
K = 16
SC = 512
MCAND = 24                      # host merge keeps top-24 candidates per row
TAU = 0.026                     # rescore-flag threshold (~8 sigma of f32r noise)
SCALE_G = -1.0 / np.sqrt(H)
ADD, MUL = mybir.AluOpType.add, mybir.AluOpType.mult


def build_launch1():
    """Per-core: g = -|supT_shard.T @ Wk.T + (bk-bq)|^2 / sqrt(H), fp32-exact.

    d-major full-width tiles: one [128, 2048] DMA per d-slice; Square rides
    the ACT engine with the bias folded in (out = Square(ps + b')); the
    cross-d accumulation is 32 wide DVE adds. PE (fp32 matmuls) is the
    critical path; everything else hides under it.
    """
    nc = bass.Bass("TRN2", target_bir_lowering=False, debug=False, num_devices=N_CORES)
    supT = nc.dram_tensor("supT", (DH, NS_SH), f32, kind="ExternalInput")
    WkT = nc.dram_tensor("WkT", (H, H), f32, kind="ExternalInput")
    bp = nc.dram_tensor("bp", (H, 1), f32, kind="ExternalInput")
    g_out = nc.dram_tensor("g", (1, NS_SH), f32, kind="ExternalOutput")

    supT_v = supT.ap().rearrange("(g p) s -> p g s", p=H)   # [128, 32, 2048]

    NCH1 = NS_SH // SC          # 4 column chunks (psum-bank sized)
    R_T, R_SQ = 6, 2

    t_sb = [nc.alloc_sbuf_tensor(f"t{i}", [H, NS_SH], f32) for i in range(R_T)]
    sq_sb = [nc.alloc_sbuf_tensor(f"sq{i}", [H, NS_SH], f32) for i in range(R_SQ)]
    sqacc = nc.alloc_sbuf_tensor("sqacc", [H, NS_SH], f32)
    WkT_sb = nc.alloc_sbuf_tensor("WkT_sb", [H, H], f32)
    bp_sb = nc.alloc_sbuf_tensor("bp_sb", [H, 1], f32)
    ones_sb = nc.alloc_sbuf_tensor("ones_sb", [H, 1], f32)
    g_sb = nc.alloc_sbuf_tensor("g_sb", [1, NS_SH], f32)

    ps = [nc.alloc_psum_tensor(f"ps{i}", [H, SC], f32) for i in range(8)]

    with (
        nc.Block() as block,
        nc.semaphore("s_const") as s_const,
        nc.semaphore("s_t0") as s_t0,
        nc.semaphore("s_t1") as s_t1,
        nc.semaphore("s_t2") as s_t2,
        nc.semaphore("s_t3") as s_t3,
        nc.semaphore("s_t4") as s_t4,
        nc.semaphore("s_t5") as s_t5,
        nc.semaphore("s_gout") as s_gout,
        nc.semaphore("pe") as pe,
        nc.semaphore("pe2") as pe2,
        nc.semaphore("act") as act,
        nc.semaphore("gam") as gam,
        nc.semaphore("av") as av,
    ):
        s_t = [s_t0, s_t1, s_t2, s_t3, s_t4, s_t5]

        @block.sync
        def _(sync):
            for src_t, sb in ((WkT, WkT_sb), (bp, bp_sb)):
                sync.dma_start(out=sb[:], in_=src_t.ap()).then_inc(s_const, 16)
            for d in range(D):
                if d >= R_T:
                    sync.wait_ge(pe, NCH1 * (d - R_T + 1))
                sync.dma_start(
                    out=t_sb[d % R_T][:], in_=supT_v[:, d, :]
                ).then_inc(s_t[d % R_T], 16)

        @block.tensor
        def _(tensor):
            tensor.wait_ge(s_const, 2 * 16)
            for d in range(D):
                tensor.wait_ge(s_t[d % R_T], 16 * (d // R_T + 1))
                if d >= 2:
                    tensor.wait_ge(act, NCH1 * (d - 1))   # bank pair freed
                for c in range(NCH1):
                    nc.tensor.matmul(
                        ps[(d % 2) * 4 + c][:], lhsT=WkT_sb[:],
                        rhs=t_sb[d % R_T][:, c * SC:(c + 1) * SC],
                        start=True, stop=True,
                    ).then_inc(pe, 1)
            tensor.wait_ge(av, D)
            for c in range(NCH1):
                nc.tensor.matmul(
                    ps[4 + c][0:1, :], lhsT=ones_sb[:],
                    rhs=sqacc[:, c * SC:(c + 1) * SC],
                    start=True, stop=True,
                ).then_inc(pe2, 1)

        @block.scalar
        def _(scalar):
            for d in range(D):
                if d >= R_SQ:
                    scalar.wait_ge(av, d - R_SQ + 1)   # sq slot consumed
                for c in range(NCH1):
                    scalar.wait_ge(pe, NCH1 * d + c + 1)
                    nc.scalar.activation(
                        sq_sb[d % R_SQ][:, c * SC:(c + 1) * SC],
                        ps[(d % 2) * 4 + c][:],
                        mybir.ActivationFunctionType.Square,
                        bias=bp_sb[:],
                    ).then_inc(act, 1)
            scalar.wait_ge(gam, NCH1)
            nc.scalar.dma_start(out=g_out.ap(), in_=g_sb[:]).then_inc(s_gout, 16)
            scalar.wait_ge(s_gout, 16)

        @block.vector
        def _(vector):
            vector.wait_ge(s_const, 2 * 16)
            nc.vector.memset(ones_sb[:], 1.0)
            for d in range(D):
                vector.wait_ge(act, NCH1 * (d + 1))
                if d == 0:
                    nc.vector.tensor_copy(
                        out=sqacc[:], in_=sq_sb[0][:]
                    ).then_inc(av, 1)
                else:
                    vector.wait_ge(av, d)
                    nc.vector.tensor_add(
                        sqacc[:], sqacc[:], sq_sb[d % R_SQ][:]
                    ).then_inc(av, 1)
            for c in range(NCH1):
                vector.wait_ge(pe2, c + 1)
                nc.vector.tensor_scalar(
                    g_sb[:, c * SC:(c + 1) * SC], ps[4 + c][0:1, :],
                    float(SCALE_G), None, MUL,
                ).then_inc(gam, 1)

    return nc


def build_launch2():
    """Per-core: 512 queries x 16384 supports, 1-pass f32r + per-chunk top-8."""
    nc = bass.Bass("TRN2", target_bir_lowering=False, debug=False, num_devices=N_CORES)
    supT = nc.dram_tensor("supT", (DH, NS), f32r, kind="ExternalInput")
    qmT = nc.dram_tensor("qmT", (DH, NQ_SH), f32r, kind="ExternalInput")
    gbc = nc.dram_tensor("gbc", (H, NS), f32, kind="ExternalInput")
    cval_out = nc.dram_tensor("cval", (4, H, 256), f32, kind="ExternalOutput")
    cidx_out = nc.dram_tensor("cidx", (4, H, 256), u16, kind="ExternalOutput")

    sup_v = supT.ap().rearrange("(g p) s -> p g s", p=H)    # [128, 32, 16384]
    qm_v = qmT.ap().rearrange("(g p) n -> p g n", p=H)      # [128, 32, 512]

    NCH2 = NS // SC             # 32 chunks
    DG = 4
    NDG = 32 // DG              # 8 sup tiles per chunk
    NT = NCH2 * NDG             # 256 sup tiles
    R_KT, R_G, R_SCB = 9, 4, 6

    qm_sb = nc.alloc_sbuf_tensor("qm_sb", [H, 32, NQ_SH], f32r)
    kt_sb = [nc.alloc_sbuf_tensor(f"kt{i}", [H, DG, SC], f32r) for i in range(R_KT)]
    g_sb = [nc.alloc_sbuf_tensor(f"gs{i}", [H, SC], f32) for i in range(R_G)]
    sc_sb = [nc.alloc_sbuf_tensor(f"scb{i}", [H, SC], f32) for i in range(R_SCB)]
    cv_sb = [nc.alloc_sbuf_tensor(f"cv{b}", [H, 256], f32) for b in range(4)]
    ci_sb = [nc.alloc_sbuf_tensor(f"ci{b}", [H, 256], u16) for b in range(4)]

    ps = [nc.alloc_psum_tensor(f"ps{i}", [H, SC], f32) for i in range(8)]

    from contextlib import ExitStack
    with ExitStack() as stack:
        block = stack.enter_context(nc.Block())
        sem = lambda name: stack.enter_context(nc.semaphore(name))
        s_qm = [sem(f"s_qm{i}") for i in range(NDG)]
        s_kt = [sem(f"s_kt{i}") for i in range(R_KT)]
        s_g = [sem(f"s_g{i}") for i in range(R_G)]
        s_out = sem("s_out")
        pe = sem("pe")
        pet = sem("pet")
        dve = sem("dve")
        tk = sem("tk")

        @block.sync
        def _(sync):
            for t in range(NT):
                c, i = t // NDG, t % NDG
                if t >= R_KT:
                    sync.wait_ge(pet, t - R_KT + 1)
                sync.dma_start(
                    out=kt_sb[t % R_KT][:],
                    in_=sup_v[:, i * DG:(i + 1) * DG, c * SC:(c + 1) * SC],
                ).then_inc(s_kt[t % R_KT], 16)

        @block.scalar
        def _(scalar):
            # qm + g tiles on the ACT HWDGE ring (parallel with the sup
            # stream on the sync ring) + final candidate output DMAs
            for i in range(NDG):
                nc.scalar.dma_start(
                    out=qm_sb[:, i * DG:(i + 1) * DG, :],
                    in_=qm_v[:, i * DG:(i + 1) * DG, :],
                ).then_inc(s_qm[i], 16)
            for c in range(NCH2):
                if c >= R_G:
                    scalar.wait_ge(dve, 4 * (c - R_G) + 4)   # slot's adds done
                nc.scalar.dma_start(
                    out=g_sb[c % R_G][:], in_=gbc.ap()[:, c * SC:(c + 1) * SC]
                ).then_inc(s_g[c % R_G], 16)
            scalar.wait_ge(tk, 8 * NCH2)
            for b in range(4):
                nc.scalar.dma_start(out=cval_out.ap()[b], in_=cv_sb[b][:]).then_inc(s_out, 16)
                nc.scalar.dma_start(out=cidx_out.ap()[b], in_=ci_sb[b][:]).then_inc(s_out, 16)
            scalar.wait_ge(s_out, 16 * 8)

        @block.tensor
        def _(tensor):
            for c in range(NCH2):
                for d in range(32):
                    i, j = d // DG, d % DG
                    t = c * NDG + i
                    if c == 0 and j == 0:
                        tensor.wait_ge(s_qm[i], 16)
                    if j == 0:
                        tensor.wait_ge(s_kt[t % R_KT], 16 * (t // R_KT + 1))
                    for b in range(4):
                        cell = 4 * c + b
                        if d == 0 and cell >= 8:
                            tensor.wait_ge(dve, cell - 8 + 1)   # psum bank freed
                        inst = nc.tensor.matmul(
                            ps[(c % 2) * 4 + b][:],
                            lhsT=qm_sb[:, d, b * H:(b + 1) * H],
                            rhs=kt_sb[t % R_KT][:, j, :],
                            start=(d == 0), stop=(d == 31),
                        )
                        # one semaphore update per instruction: b0..b2 stops
                        # mark pe (3/chunk); the b3/d31 stop marks its tile's
                        # pet (which also implies the whole chunk finished).
                        if d == 31 and b < 3:
                            inst.then_inc(pe, 1)
                        elif j == DG - 1 and b == 3:
                            inst.then_inc(pet, 1)   # sup tile fully consumed

        @block.vector
        def _(vector):
            for c in range(NCH2):
                vector.wait_ge(s_g[c % R_G], 16 * (c // R_G + 1))
                for b in range(4):
                    cell = 4 * c + b
                    if b < 3:
                        vector.wait_ge(pe, 3 * c + b + 1)
                    else:
                        vector.wait_ge(pet, NDG * (c + 1))
                    nc.vector.tensor_tensor(
                        out=sc_sb[cell % R_SCB][:],
                        in0=ps[(c % 2) * 4 + b][:], in1=g_sb[c % R_G][:], op=ADD,
                    ).then_inc(dve, 1)
                    vector.wait_ge(dve, cell + 1)       # same-engine RAW
                    nc.vector.max(
                        out=cv_sb[b][:, c * 8:c * 8 + 8],
                        in_=sc_sb[cell % R_SCB][:],
                    ).then_inc(tk, 1)
                    vector.wait_ge(tk, 2 * cell + 1)    # same-engine RAW
                    nc.vector.max_index(
                        out=ci_sb[b][:, c * 8:c * 8 + 8],
                        in_max=cv_sb[b][:, c * 8:c * 8 + 8],
                        in_values=sc_sb[cell % R_SCB][:],
                    ).then_inc(tk, 1)

    return nc


_CACHE = {}


def _get_programs():
    if "l1" not in _CACHE:
        _CACHE["l1"] = build_launch1()
        _CACHE["l2"] = build_launch2()
    return _CACHE["l1"], _CACHE["l2"]


def run_launches(query, support, Wq, bq, Wk, bk, trace2=False, trace1=False):
    nc1, nc2 = _get_programs()

    sflat = np.ascontiguousarray(support.reshape(NS, DH))
    supT = np.ascontiguousarray(sflat.T)
    WkT_a = np.ascontiguousarray(Wk.T)
    bp = np.ascontiguousarray((bk - bq).reshape(H, 1))

    in_maps1 = [
        {
            "supT": np.ascontiguousarray(supT[:, c * NS_SH:(c + 1) * NS_SH]),
            "WkT": WkT_a, "bp": bp,
        }
        for c in range(N_CORES)
    ]
    res1 = run_bass_kernel_spmd(
        nc1, in_maps1, core_ids=list(range(N_CORES)), trace=trace1
    )
    gvec = np.concatenate([res1.results[c]["g"][0] for c in range(N_CORES)])

    M = ((Wq.T @ Wk) * np.float32(2.0 / np.sqrt(H))).astype(np.float32)
    qm = (query.reshape(NQ * D, H) @ M).reshape(NQ, DH)
    gbc_a = np.ascontiguousarray(np.broadcast_to(gvec, (H, NS)))

    in_maps2 = [
        {
            "supT": supT,
            "qmT": np.ascontiguousarray(qm[c * NQ_SH:(c + 1) * NQ_SH].T),
            "gbc": gbc_a,
        }
        for c in range(N_CORES)
    ]
    res2 = run_bass_kernel_spmd(
        nc2, in_maps2, core_ids=list(range(N_CORES)), trace=trace2
    )

    # ---- host merge: (4, H, 256) per core -> (NQ, 256) candidate vals/idx
    NCH2 = NS // SC
    cvals = np.empty((NQ, NCH2 * 8), np.float32)
    cidx = np.empty((NQ, NCH2 * 8), np.int64)
    base = (np.arange(NCH2, dtype=np.int64) * SC).repeat(8)[None, :]
    for c in range(N_CORES):
        cv = res2.results[c]["cval"].reshape(4 * H, NCH2 * 8)
        ci = res2.results[c]["cidx"].reshape(4 * H, NCH2 * 8).astype(np.int64)
        cvals[c * NQ_SH:(c + 1) * NQ_SH] = cv
        cidx[c * NQ_SH:(c + 1) * NQ_SH] = ci + base

    part = np.argpartition(-cvals, MCAND, 1)[:, :MCAND]
    pv = np.take_along_axis(cvals, part, 1)
    pi = np.take_along_axis(cidx, part, 1)
    order = np.lexsort((pi, -pv), axis=1)
    pv = np.take_along_axis(pv, order, 1)
    pi = np.take_along_axis(pi, order, 1)

    # flag rows whose top-17 adjacent gaps could be reordered by f32r noise
    flag = ((pv[:, :16] - pv[:, 1:17]) < TAU).any(1)
    fr = np.where(flag)[0]
    if fr.size:
        sel = sflat[pi[fr].ravel()].reshape(fr.size, MCAND, DH)
        ex = np.einsum(
            "nd,ncd->nc", qm[fr], sel, dtype=np.float64, optimize=True
        ) + gvec[pi[fr]]
        o2 = np.lexsort((pi[fr], -ex), axis=1)
        pv[fr] = np.take_along_axis(ex, o2, 1).astype(np.float32)
        pi[fr] = np.take_along_axis(pi[fr], o2, 1)

    idx = pi[:, :16].astype(np.int32)
    tv = pv[:, :16].astype(np.float64)
    e = np.exp(tv - tv[:, :1])
    w = (e / e.sum(1, keepdims=True)).astype(np.float32)
    return idx, w, (res1, res2)


def kernel(query, support, Wq, bq, Wk, bk, k):
    assert int(k) == K
    query = np.asarray(query, np.float32)
    support = np.asarray(support, np.float32)
    Wq = np.asarray(Wq, np.float32)
    bq = np.asarray(bq, np.float32)
    Wk = np.asarray(Wk, np.float32)
    bk = np.asarray(bk, np.float32)
    idx, w, _ = run_launches(query, support, Wq, bq, Wk, bk)
    return idx, w


# revision 16
# speedup vs baseline: 1.0119x; 1.0119x over previous
"""AttentionRetrieval kNN kernel for 8 TRN2 NeuronCores (Bass, raw Block style).

Reference math:
    qp  = query @ Wq.T + bq           (4096, 4096)   [flattened over (D=32, H=128)]
    kp  = support @ Wk.T + bk         (16384, 4096)
    sim = -(|qp|^2 + |kp|^2 - 2 qp@kp.T) / sqrt(128)
    idx, w = top16(sim), softmax(top16 values)

Fused formulation (per-row constants drop out of topk and softmax):
    score[i,j] = sum_d (q_d M) s_d^T [i,j] + g[j]
      M  = (2/sqrt(H)) Wq^T Wk                  (queries projected once, host)
      g  = -|s Wk^T + (bk - bq)|^2 / sqrt(H)    (completed square folds the
                                                 bq-cross-term; global consts drop)
so launch 2 streams the RAW transposed support once — no kpT materialization.

Launch 1 (support sharded 8 x 2048): fp32 (exact) projection with bias
(bk - bq), square + column-sum -> g shard (1 x 2048, 8 KB out per core).

Launch 2 (queries sharded 8 x 512): single-pass float32r matmul
(qm_d stationary, raw supT moving; f32r = RNE-11-bit input rounding at
1 cycle/row — 3x fewer PE rows than an exact hi/lo scheme), + g add, and
per-512-chunk top-8 (DVE max8 + max_index) -> 256 candidates/row.

Host: merge 256 candidates -> top-24, flag rows whose top-17 adjacent gaps
are below the f32r noise bound, exactly rescore flagged rows in f64
(24 dot products each), then top-16 + softmax. Flip rate vs the fp32
reference matches an exact device kernel (~2 rows from fp32 tie noise).
"""
import sys
sys.path.insert(0, "/opt/trn_rl_repo")
import numpy as np
import concourse.bass as bass
from concourse import mybir
from concourse.bass_utils import run_bass_kernel_spmd

f32 = mybir.dt.float32
f32r = mybir.dt.float32r
u16 = mybir.dt.uint16

N_CORES = 8
NQ, NS, D, H = 4096, 16384, 32, 128
DH = D * H
NQ_SH = NQ // N_CORES           # 512
NS_SH = NS // N_CORES           # 2048
K = 16
SC = 512
MCAND = 24                      # host merge keeps top-24 candidates per row
TAU = 0.026                     # rescore-flag threshold (~8 sigma of f32r noise)
SCALE_G = -1.0 / np.sqrt(H)
ADD, MUL = mybir.AluOpType.add, mybir.AluOpType.mult


def build_launch1():
    """Per-core: g = -|supT_shard.T @ Wk.T + (bk-bq)|^2 / sqrt(H), fp32-exact.

    d-major full-width tiles: one [128, 2048] DMA per d-slice; Square rides
    the ACT engine with the bias folded in (out = Square(ps + b')); the
    cross-d accumulation is 32 wide DVE adds. PE (fp32 matmuls) is the
    critical path; everything else hides under it.
    """
    nc = bass.Bass("TRN2", target_bir_lowering=False, debug=False, num_devices=N_CORES)
    supT = nc.dram_tensor("supT", (DH, NS_SH), f32, kind="ExternalInput")
    WkT = nc.dram_tensor("WkT", (H, H), f32, kind="ExternalInput")
    bp = nc.dram_tensor("bp", (H, 1), f32, kind="ExternalInput")
    g_out = nc.dram_tensor("g", (1, NS_SH), f32, kind="ExternalOutput")

    supT_v = supT.ap().rearrange("(g p) s -> p g s", p=H)   # [128, 32, 2048]

    NCH1 = NS_SH // SC          # 4 column chunks (psum-bank sized)
    R_T, R_SQ = 6, 2

    t_sb = [nc.alloc_sbuf_tensor(f"t{i}", [H, NS_SH], f32) for i in range(R_T)]
    sq_sb = [nc.alloc_sbuf_tensor(f"sq{i}", [H, NS_SH], f32) for i in range(R_SQ)]
    sqacc = nc.alloc_sbuf_tensor("sqacc", [H, NS_SH], f32)
    WkT_sb = nc.alloc_sbuf_tensor("WkT_sb", [H, H], f32)
    bp_sb = nc.alloc_sbuf_tensor("bp_sb", [H, 1], f32)
    ones_sb = nc.alloc_sbuf_tensor("ones_sb", [H, 1], f32)
    g_sb = nc.alloc_sbuf_tensor("g_sb", [1, NS_SH], f32)

    ps = [nc.alloc_psum_tensor(f"ps{i}", [H, SC], f32) for i in range(8)]

    with (
        nc.Block() as block,
        nc.semaphore("s_const") as s_const,
        nc.semaphore("s_t0") as s_t0,
        nc.semaphore("s_t1") as s_t1,
        nc.semaphore("s_t2") as s_t2,
        nc.semaphore("s_t3") as s_t3,
        nc.semaphore("s_t4") as s_t4,
        nc.semaphore("s_t5") as s_t5,
        nc.semaphore("s_gout") as s_gout,
        nc.semaphore("pe") as pe,
        nc.semaphore("pe2") as pe2,
        nc.semaphore("act") as act,
        nc.semaphore("gam") as gam,
        nc.semaphore("av") as av,
    ):
        s_t = [s_t0, s_t1, s_t2, s_t3, s_t4, s_t5]

        @block.sync
        def _(sync):
            for src_t, sb in ((WkT, WkT_sb), (bp, bp_sb)):
                sync.dma_start(out=sb[:], in_=src_t.ap()).then_inc(s_const, 16)
            for d in range(D):
                if d >= R_T:
                    sync.wait_ge(pe, NCH1 * (d - R_T + 1))
                sync.dma_start(
                    out=t_sb[d % R_T][:], in_=supT_v[:, d, :]
                ).then_inc(s_t[d % R_T], 16)

        @block.tensor
        def _(tensor):
            tensor.wait_ge(s_const, 2 * 16)
            for d in range(D):
                tensor.wait_ge(s_t[d % R_T], 16 * (d // R_T + 1))
                if d >= 2:
                    tensor.wait_ge(act, NCH1 * (d - 1))   # bank pair freed
                for c in range(NCH1):
                    nc.tensor.matmul(
                        ps[(d % 2) * 4 + c][:], lhsT=WkT_sb[:],
                        rhs=t_sb[d % R_T][:, c * SC:(c + 1) * SC],
                        start=True, stop=True,
                    ).then_inc(pe, 1)
            tensor.wait_ge(av, D)
            for c in range(NCH1):
                nc.tensor.matmul(
                    ps[4 + c][0:1, :], lhsT=ones_sb[:],
                    rhs=sqacc[:, c * SC:(c + 1) * SC],
                    start=True, stop=True,
                ).then_inc(pe2, 1)

        @block.scalar
        def _(scalar):
            for d in range(D):
                if d >= R_SQ:
                    scalar.wait_ge(av, d - R_SQ + 1)   # sq slot consumed
                for c in range(NCH1):
                    scalar.wait_ge(pe, NCH1 * d + c + 1)
                    nc.scalar.activation(
                        sq_sb[d % R_SQ][:, c * SC:(c + 1) * SC],
                        ps[(d % 2) * 4 + c][:],
                        mybir.ActivationFunctionType.Square,
                        bias=bp_sb[:],
                    ).then_inc(act, 1)
            scalar.wait_ge(gam, NCH1)
            nc.scalar.dma_start(out=g_out.ap(), in_=g_sb[:]).then_inc(s_gout, 16)
            scalar.wait_ge(s_gout, 16)

        @block.vector
        def _(vector):
            vector.wait_ge(s_const, 2 * 16)
            nc.vector.memset(ones_sb[:], 1.0)
            for d in range(D):
                vector.wait_ge(act, NCH1 * (d + 1))
                if d == 0:
                    nc.vector.tensor_copy(
                        out=sqacc[:], in_=sq_sb[0][:]
                    ).then_inc(av, 1)
                else:
                    vector.wait_ge(av, d)
                    nc.vector.tensor_add(
                        sqacc[:], sqacc[:], sq_sb[d % R_SQ][:]
                    ).then_inc(av, 1)
            for c in range(NCH1):
                vector.wait_ge(pe2, c + 1)
                nc.vector.tensor_scalar(
                    g_sb[:, c * SC:(c + 1) * SC], ps[4 + c][0:1, :],
                    float(SCALE_G), None, MUL,
                ).then_inc(gam, 1)

    return nc


def build_launch2():
    """Per-core: 512 queries x 16384 supports, 1-pass f32r + per-chunk top-8."""
    nc = bass.Bass("TRN2", target_bir_lowering=False, debug=False, num_devices=N_CORES)
    supT = nc.dram_tensor("supT", (DH, NS), f32r, kind="ExternalInput")
    qmT = nc.dram_tensor("qmT", (DH, NQ_SH), f32r, kind="ExternalInput")
    gbc = nc.dram_tensor("gbc", (H, NS), f32, kind="ExternalInput")
    cval_out = nc.dram_tensor("cval", (4, H, 256), f32, kind="ExternalOutput")
    cidx_out = nc.dram_tensor("cidx", (4, H, 256), u16, kind="ExternalOutput")

    sup_v = supT.ap().rearrange("(g p) s -> p g s", p=H)    # [128, 32, 16384]
    qm_v = qmT.ap().rearrange("(g p) n -> p g n", p=H)      # [128, 32, 512]

    NCH2 = NS // SC             # 32 chunks
    DG = 4
    NDG = 32 // DG              # 8 sup tiles per chunk
    NT = NCH2 * NDG             # 256 sup tiles
    R_KT, R_G, R_SCB = 10, 4, 6

    qm_sb = nc.alloc_sbuf_tensor("qm_sb", [H, 32, NQ_SH], f32r)
    kt_sb = [nc.alloc_sbuf_tensor(f"kt{i}", [H, DG, SC], f32r) for i in range(R_KT)]
    g_sb = [nc.alloc_sbuf_tensor(f"gs{i}", [H, SC], f32) for i in range(R_G)]
    sc_sb = [nc.alloc_sbuf_tensor(f"scb{i}", [H, SC], f32) for i in range(R_SCB)]
    cv_sb = [nc.alloc_sbuf_tensor(f"cv{b}", [H, 256], f32) for b in range(4)]
    ci_sb = [nc.alloc_sbuf_tensor(f"ci{b}", [H, 256], u16) for b in range(4)]

    ps = [nc.alloc_psum_tensor(f"ps{i}", [H, SC], f32) for i in range(8)]

    from contextlib import ExitStack
    with ExitStack() as stack:
        block = stack.enter_context(nc.Block())
        sem = lambda name: stack.enter_context(nc.semaphore(name))
        s_qm = [sem(f"s_qm{i}") for i in range(NDG)]
        s_kt = [sem(f"s_kt{i}") for i in range(R_KT)]
        s_g = [sem(f"s_g{i}") for i in range(R_G)]
        s_out = sem("s_out")
        pe = sem("pe")
        pet = sem("pet")
        dve = sem("dve")
        tk = sem("tk")

        @block.sync
        def _(sync):
            for t in range(NT):
                c, i = t // NDG, t % NDG
                if t >= R_KT:
                    sync.wait_ge(pet, t - R_KT + 1)
                sync.dma_start(
                    out=kt_sb[t % R_KT][:],
                    in_=sup_v[:, i * DG:(i + 1) * DG, c * SC:(c + 1) * SC],
                ).then_inc(s_kt[t % R_KT], 16)

        @block.scalar
        def _(scalar):
            # qm + g tiles on the ACT HWDGE ring (parallel with the sup
            # stream on the sync ring) + final candidate output DMAs
            for i in range(NDG):
                nc.scalar.dma_start(
                    out=qm_sb[:, i * DG:(i + 1) * DG, :],
                    in_=qm_v[:, i * DG:(i + 1) * DG, :],
                ).then_inc(s_qm[i], 16)
            for c in range(NCH2):
                if c >= R_G:
                    scalar.wait_ge(dve, 4 * (c - R_G) + 4)   # slot's adds done
                nc.scalar.dma_start(
                    out=g_sb[c % R_G][:], in_=gbc.ap()[:, c * SC:(c + 1) * SC]
                ).then_inc(s_g[c % R_G], 16)
            scalar.wait_ge(tk, 8 * NCH2)
            for b in range(4):
                nc.scalar.dma_start(out=cval_out.ap()[b], in_=cv_sb[b][:]).then_inc(s_out, 16)
                nc.scalar.dma_start(out=cidx_out.ap()[b], in_=ci_sb[b][:]).then_inc(s_out, 16)
            scalar.wait_ge(s_out, 16 * 8)

        @block.tensor
        def _(tensor):
            for c in range(NCH2):
                for d in range(32):
                    i, j = d // DG, d % DG
                    t = c * NDG + i
                    if c == 0 and j == 0:
                        tensor.wait_ge(s_qm[i], 16)
                    if j == 0:
                        tensor.wait_ge(s_kt[t % R_KT], 16 * (t // R_KT + 1))
                    for b in range(4):
                        cell = 4 * c + b
                        if d == 0 and cell >= 8:
                            tensor.wait_ge(dve, cell - 8 + 1)   # psum bank freed
                        inst = nc.tensor.matmul(
                            ps[(c % 2) * 4 + b][:],
                            lhsT=qm_sb[:, d, b * H:(b + 1) * H],
                            rhs=kt_sb[t % R_KT][:, j, :],
                            start=(d == 0), stop=(d == 31),
                        )
                        # one semaphore update per instruction: b0..b2 stops
                        # mark pe (3/chunk); the b3/d31 stop marks its tile's
                        # pet (which also implies the whole chunk finished).
                        if d == 31 and b < 3:
                            inst.then_inc(pe, 1)
                        elif j == DG - 1 and b == 3:
                            inst.then_inc(pet, 1)   # sup tile fully consumed

        @block.vector
        def _(vector):
            for c in range(NCH2):
                vector.wait_ge(s_g[c % R_G], 16 * (c // R_G + 1))
                for b in range(4):
                    cell = 4 * c + b
                    if b < 3:
                        vector.wait_ge(pe, 3 * c + b + 1)
                    else:
                        vector.wait_ge(pet, NDG * (c + 1))
                    nc.vector.tensor_tensor(
                        out=sc_sb[cell % R_SCB][:],
                        in0=ps[(c % 2) * 4 + b][:], in1=g_sb[c % R_G][:], op=ADD,
                    ).then_inc(dve, 1)
                    vector.wait_ge(dve, cell + 1)       # same-engine RAW
                    nc.vector.max(
                        out=cv_sb[b][:, c * 8:c * 8 + 8],
                        in_=sc_sb[cell % R_SCB][:],
                    ).then_inc(tk, 1)
                    vector.wait_ge(tk, 2 * cell + 1)    # same-engine RAW
                    nc.vector.max_index(
                        out=ci_sb[b][:, c * 8:c * 8 + 8],
                        in_max=cv_sb[b][:, c * 8:c * 8 + 8],
                        in_values=sc_sb[cell % R_SCB][:],
                    ).then_inc(tk, 1)

    return nc


_CACHE = {}


def _get_programs():
    if "l1" not in _CACHE:
        _CACHE["l1"] = build_launch1()
        _CACHE["l2"] = build_launch2()
    return _CACHE["l1"], _CACHE["l2"]


def run_launches(query, support, Wq, bq, Wk, bk, trace2=False, trace1=False):
    nc1, nc2 = _get_programs()

    sflat = np.ascontiguousarray(support.reshape(NS, DH))
    supT = np.ascontiguousarray(sflat.T)
    WkT_a = np.ascontiguousarray(Wk.T)
    bp = np.ascontiguousarray((bk - bq).reshape(H, 1))

    in_maps1 = [
        {
            "supT": np.ascontiguousarray(supT[:, c * NS_SH:(c + 1) * NS_SH]),
            "WkT": WkT_a, "bp": bp,
        }
        for c in range(N_CORES)
    ]
    res1 = run_bass_kernel_spmd(
        nc1, in_maps1, core_ids=list(range(N_CORES)), trace=trace1
    )
    gvec = np.concatenate([res1.results[c]["g"][0] for c in range(N_CORES)])

    M = ((Wq.T @ Wk) * np.float32(2.0 / np.sqrt(H))).astype(np.float32)
    qm = (query.reshape(NQ * D, H) @ M).reshape(NQ, DH)
    gbc_a = np.ascontiguousarray(np.broadcast_to(gvec, (H, NS)))

    in_maps2 = [
        {
            "supT": supT,
            "qmT": np.ascontiguousarray(qm[c * NQ_SH:(c + 1) * NQ_SH].T),
            "gbc": gbc_a,
        }
        for c in range(N_CORES)
    ]
    res2 = run_bass_kernel_spmd(
        nc2, in_maps2, core_ids=list(range(N_CORES)), trace=trace2
    )

    # ---- host merge: (4, H, 256) per core -> (NQ, 256) candidate vals/idx
    NCH2 = NS // SC
    cvals = np.empty((NQ, NCH2 * 8), np.float32)
    cidx = np.empty((NQ, NCH2 * 8), np.int64)
    base = (np.arange(NCH2, dtype=np.int64) * SC).repeat(8)[None, :]
    for c in range(N_CORES):
        cv = res2.results[c]["cval"].reshape(4 * H, NCH2 * 8)
        ci = res2.results[c]["cidx"].reshape(4 * H, NCH2 * 8).astype(np.int64)
        cvals[c * NQ_SH:(c + 1) * NQ_SH] = cv
        cidx[c * NQ_SH:(c + 1) * NQ_SH] = ci + base

    part = np.argpartition(-cvals, MCAND, 1)[:, :MCAND]
    pv = np.take_along_axis(cvals, part, 1)
    pi = np.take_along_axis(cidx, part, 1)
    order = np.lexsort((pi, -pv), axis=1)
    pv = np.take_along_axis(pv, order, 1)
    pi = np.take_along_axis(pi, order, 1)

    # flag rows whose top-17 adjacent gaps could be reordered by f32r noise
    flag = ((pv[:, :16] - pv[:, 1:17]) < TAU).any(1)
    fr = np.where(flag)[0]
    if fr.size:
        sel = sflat[pi[fr].ravel()].reshape(fr.size, MCAND, DH)
        ex = np.einsum(
            "nd,ncd->nc", qm[fr], sel, dtype=np.float64, optimize=True
        ) + gvec[pi[fr]]
        o2 = np.lexsort((pi[fr], -ex), axis=1)
        pv[fr] = np.take_along_axis(ex, o2, 1).astype(np.float32)
        pi[fr] = np.take_along_axis(pi[fr], o2, 1)

    idx = pi[:, :16].astype(np.int32)
    tv = pv[:, :16].astype(np.float64)
    e = np.exp(tv - tv[:, :1])
    w = (e / e.sum(1, keepdims=True)).astype(np.float32)
    return idx, w, (res1, res2)


def kernel(query, support, Wq, bq, Wk, bk, k):
    assert int(k) == K
    query = np.asarray(query, np.float32)
    support = np.asarray(support, np.float32)
    Wq = np.asarray(Wq, np.float32)
    bq = np.asarray(bq, np.float32)
    Wk = np.asarray(Wk, np.float32)
    bk = np.asarray(bk, np.float32)
    idx, w, _ = run_launches(query, support, Wq, bq, Wk, bk)
    return idx, w
